# revision 19
# baseline (speedup 1.0000x reference)
"""Sparse Bahdanau attention kernel for Trainium2 (8 NeuronCores, data-parallel
over batch).

Shapes (hardcoded): B=32, S=2048, H=1024, QS=1024, VS=2048. Per core: 4 batches.

Math per batch b:
  q = query[b] @ Wq                                  # [H]
  scores[s] = sum_h v_energy[h] * tanh(q[h] + proj_key[b,s,h])
  alphas = softmax(scores masked by mask[b])         # [S]
  context = alphas @ value[b]                        # [VS]

Sparsity: mask[b,s] ~ Bernoulli(1/2); masked positions have alphas == 0
exactly, so their proj_key/value rows are never needed. Host-side prep (pure
numpy, part of input sharding) compacts each batch's unmasked positions into a
padded index list of NPAD=1152 slots (pad = duplicate of the first valid row),
and the device gathers ONLY those rows via SWDGE dma_gather (4KB/8KB rows, row
granularity is DMA-efficient). All math runs in the compacted domain; padded
slots are zeroed in the softmax numerator via a validity mask, making their
context contribution exactly 0. The device emits alphas in the compacted
layout; the host unshard step places them at their full-domain positions
(masked positions are exactly 0). indirect_dma_start is avoided entirely —
it wedges the device on this runtime (probed in isolation); dma_gather via
the mlp gpsimd library is fine.

This roughly halves HBM traffic vs the dense kernel (per core: ~4MB Wq +
4x1152x4KB proj_key + 4x1152x8KB value ~= 59MB vs 101MB dense).

Device strategy per core (per batch):
  - pk gather: one dma_gather of NPAD rows -> [128, 9, H] (slot i ->
    partition i%128, column i//128).
  - scores: DVE add (q broadcast) -> ACT tanh -> DVE mul (v_energy) -> ACT
    Identity with free-dim accumulator -> scores [128, 9]. Softmax without max
    subtraction (|scores| <= ||v_energy||_1 ~ 26), validity mask applied
    multiplicatively, cross-partition sum + broadcast of Z via ones matmul.
  - value gather: 3 chunked dma_gathers of 384 rows -> [128, 3, VS] f32r
    (bitcast; rounding happens in the PE), interleaved with the context
    matmuls (alphas-stationary, psum [1, VS], accumulated over 9 columns).
  - alphas: indirect_dma_start elementwise scatter into the zeroed full
    output; q computed on PE as in the dense kernel (DRAM-bounce broadcast).
  - A post-pass splits multi-wait instructions into chains of single-wait
    NOPs (this walrus build fits only one sync-wait per instruction).
"""

import numpy as np

import concourse.bass as bass
import concourse.tile as tile
from concourse import library_config, mybir
from concourse.bass_utils import run_bass_kernel_spmd

B, S, H, QS, VS = 32, 2048, 1024, 1024, 2048
NCORES = 8
BPC = B // NCORES  # batches per core

F32 = mybir.dt.float32
F32R = mybir.dt.float32r
I32 = mybir.dt.int32
I16 = mybir.dt.int16

HC = H // 128        # 8 h-chunks (for Wq/queryT layout)
NPAD = 1152          # compacted slots per batch (multiple of 128; counts ~1032)
NCOL = NPAD // 128   # 9 column-chunks in the compacted layout
NW = NPAD // 16      # 72 wrapped int16 index columns (dma_gather layout)
VCH = 384            # value-gather chunk (rows)
VCC = VCH // 128     # 3 columns per value chunk
NVG = NPAD // VCH    # 3 value gathers per batch
VW = VCH // 16       # 24 wrapped index columns per value chunk

USE_F32R_CTX = True  # fallback to exact fp32 context matmul if False

# Only these instruction types get their excess waits split onto NOPs —
# raw-encoded (InstISA) and sync-machinery instructions are left exactly
# as Tile emitted them.
_SPLIT_TYPES = (
    "InstMatmult",
    "InstDMACopy",
    "InstDMAGatherAnt",
    "InstActivation",
    "InstTensorCopy",
    "InstTensorTensor",
    "InstTensorReduce",
    "InstTensorScalarPtr",
    "InstMemset",
    "InstReciprocal",
    "InstLdweights",
    "InstDrain",
    "InstEventSemaphore",
    "InstNoOp",
)


def _make_wait_nop(nc, engine_type, wait):
    """Build a properly-encoded NOP via the engine API (it lands at the
    tail of the current bb), detach it, and give it the single wait
    (encoded through the proper wait_op path)."""
    import bass_rust as _br

    bi = nc.engines[engine_type].nop(nofuse=True)
    sem = _br.SemaphoreHandle(wait.ant_name or f"sem{wait.id}", wait.id)
    bi._wait_ge(sem, wait.wait_value)
    ni = bi.ins
    for fn in nc.m.functions:
        for blk in fn.blocks:
            if blk.instructions and blk.instructions[-1].name == ni.name:
                lst = list(blk.instructions)
                lst.pop()
                blk.instructions = lst
                return ni
    raise RuntimeError("freshly added nop not found at any block tail")


def _split_excess_waits(nc):
    """This walrus build fits only ONE sync-wait into most instruction
    encodings ("Too many sync wait commands" codegen errors). Move every
    wait beyond the first onto standalone same-engine NOPs inserted right
    before the instruction — the sequencer waits on each in turn, which is
    semantically identical."""
    for fn in nc.m.functions:
        for blk in fn.blocks:
            offenders = [
                inst
                for inst in blk.instructions
                if inst.sync_info is not None
                and inst.sync_info.on_wait
                and len(inst.sync_info.on_wait) > 1
                and type(inst).__name__ in _SPLIT_TYPES
            ]
            if not offenders:
                continue
            pre = {}
            for inst in offenders:
                si = inst.sync_info
                waits = list(si.on_wait)
                pre[inst.name] = [
                    _make_wait_nop(nc, inst.engine, w) for w in waits[:-1]
                ]
                inst.sync_info = mybir.SyncInfo(
                    on_wait=[waits[-1]],
                    on_update=list(si.on_update) if si.on_update else [],
                )
            out = []
            for inst in blk.instructions:
                out.extend(pre.get(inst.name, ()))
                out.append(inst)
            blk.instructions = out
    return nc


def _ap(t, offset, dims):
    return bass.AP(tensor=t, offset=offset, ap=[list(d) for d in dims])


def build_nc():
    nc = bass.Bass()

    query = nc.dram_tensor("query", [BPC, QS], F32, kind="ExternalInput")
    pk = nc.dram_tensor("proj_key", [BPC, S, H], F32, kind="ExternalInput")
    value = nc.dram_tensor("value", [BPC, S, VS], F32, kind="ExternalInput")
    wq = nc.dram_tensor("Wq", [QS, H], F32, kind="ExternalInput")
    ve = nc.dram_tensor("v_energy", [H], F32, kind="ExternalInput")
    gidx = nc.dram_tensor("gidx", [BPC, 128, NW], I16, kind="ExternalInput")
    vmask = nc.dram_tensor("vmask", [BPC, 128, NCOL], F32, kind="ExternalInput")
    ctx_out = nc.dram_tensor("context", [BPC, 1, VS], F32, kind="ExternalOutput")
    # compacted alphas [batch, partition, column]; host places them at their
    # full-domain positions during unshard
    al_out = nc.dram_tensor("alphas_c", [BPC, 128, NCOL], F32,
                            kind="ExternalOutput")

    vdt = F32R if USE_F32R_CTX else F32

    with tile.TileContext(nc) as tc:
        with (
            tc.tile_pool(name="consts", bufs=1) as consts,
            tc.tile_pool(name="qb", bufs=1) as qbp,
            tc.tile_pool(name="dramp", bufs=1, space="DRAM") as dramp,
        ):
            # the dma_gather ucode lives in the mlp gpsimd library
            nc.gpsimd.load_library(library_config.mlp)

            # ---- constants / prologue ----
            ve_bcast = consts.tile([128, H], F32)
            nc.gpsimd.dma_start(out=ve_bcast, in_=_ap(ve, 0, [[0, 128], [1, H]]))

            ones128 = consts.tile([128, 128], F32)
            nc.vector.memset(ones128, 1.0)

            idx_t = consts.tile([128, BPC, NW], I16)
            nc.sync.dma_start(
                out=idx_t, in_=_ap(gidx, 0, [[NW, 128], [128 * NW, BPC], [1, NW]])
            )
            vmask_t = consts.tile([128, BPC, NCOL], F32)
            nc.sync.dma_start(
                out=vmask_t,
                in_=_ap(vmask, 0, [[NCOL, 128], [128 * NCOL, BPC], [1, NCOL]]),
            )

            # ---- main loop over batches (compacted domain) ----
            # the q computation lives INSIDE the streaming scope, issued
            # after the first gathers: the Pool engine stream must reach the
            # batch-0/1 gather issues before it blocks on the q chain, or
            # the DMA queue idles ~15us at the start
            with (
                tc.tile_pool(name="pkp", bufs=2) as pkp,
                tc.tile_pool(name="wqp", bufs=1) as wqp,
                tc.tile_pool(name="tp", bufs=2) as tp,
                tc.tile_pool(name="vp", bufs=3) as vp,
                tc.tile_pool(name="sm", bufs=2) as sm,
                tc.tile_pool(name="psc", bufs=2, space="PSUM") as psc,
            ):
                pk_src = _ap(pk, 0, [[H, BPC * S], [1, H]])
                v_src = _ap(value, 0, [[VS, BPC * S], [1, VS]])
                if USE_F32R_CTX:
                    v_src = v_src.bitcast(F32R)

                # dma_gather descriptor streams must stay under the SWDGE
                # ring capacity (1152-row gathers hang the device; 384-row
                # chunks are safe) -> gather pk in 3 chunks like value
                def gather_pk(pk_t, bb):
                    for g in range(NVG):
                        nc.gpsimd.dma_gather(
                            pk_t[:, g * VCC : (g + 1) * VCC, :],
                            pk_src,
                            idx_t[:, bb, g * VW : (g + 1) * VW],
                            VCH,
                            VCH,
                            H,
                        )

                def gather_v(bb):
                    v_tiles = []
                    for g in range(NVG):
                        v_t = vp.tile([128, VCC, VS], vdt, tag="v", name="v_t")
                        nc.gpsimd.dma_gather(
                            v_t,
                            v_src,
                            idx_t[:, bb, g * VW : (g + 1) * VW],
                            VCH,
                            VCH,
                            VS,
                        )
                        v_tiles.append(v_t)
                    return v_tiles

                # prefetch: pk(0), value(0), pk(1) hit the queue immediately
                pk_tiles = [None] * BPC
                pk_tiles[0] = pkp.tile([128, NCOL, H], F32, tag="pk", name="pk_c0")
                gather_pk(pk_tiles[0], 0)
                v_tiles_0 = gather_v(0)
                pk_tiles[1] = pkp.tile([128, NCOL, H], F32, tag="pk", name="pk_c1")
                gather_pk(pk_tiles[1], 1)

                # ---- q = query @ Wq prologue (overlaps the prefetched
                # gathers; Wq loaded in 256-col quarters to fit SBUF, psum
                # shared with the ctx pool) ----
                q_sb = consts.tile([BPC, H], F32)
                qT = wqp.tile([128, HC, BPC], F32, tag="qT")
                for j in range(HC):
                    nc.sync.dma_start(
                        out=qT[:, j, :],
                        in_=_ap(query, j * 128, [[1, 128], [QS, BPC]]),
                    )
                qp = psc.tile([128, VS], F32, tag="ctx", name="q_psum")
                for quarter in range(4):
                    wq_t = wqp.tile([128, HC, 256], F32, tag="wq", name="wq_t")
                    nc.sync.dma_start(
                        out=wq_t,
                        in_=_ap(
                            wq,
                            quarter * 256,
                            [[H, 128], [128 * H, HC], [1, 256]],
                        ),
                    )
                    for j in range(HC):
                        nc.tensor.matmul(
                            out=qp[0:BPC, quarter * 256 : (quarter + 1) * 256],
                            lhsT=qT[:, j, :],
                            rhs=wq_t[:, j, :],
                            start=(j == 0),
                            stop=(j == HC - 1),
                            skip_group_check=True,
                        )
                nc.vector.tensor_copy(out=q_sb, in_=qp[0:BPC, 0:H])

                # broadcast q[b] across 128 partitions via DRAM bounce +
                # partition-stride-0 load
                q_dram = dramp.tile([BPC, H], F32)
                nc.gpsimd.dma_start(out=q_dram, in_=q_sb)
                q_bc = []
                for b in range(BPC):
                    qb_t = qbp.tile([128, H], F32, tag=f"qbc{b}")
                    nc.gpsimd.dma_start(
                        out=qb_t, in_=_ap(q_dram.tensor, b * H, [[0, 128], [1, H]])
                    )
                    q_bc.append(qb_t)

                pending_ctx = [None] * BPC
                for bb in range(BPC):
                    pk_c = pk_tiles[bb]

                    # value gathers for this batch go out first: they only
                    # depend on the index tile + a free pool buffer, so the
                    # DMA queue stays busy under the scores compute
                    v_tiles = v_tiles_0 if bb == 0 else gather_v(bb)

                    # prefetch next batch's pk gather (batch 1 already done)
                    if bb + 1 < BPC and bb > 0:
                        pk_tiles[bb + 1] = pkp.tile(
                            [128, NCOL, H], F32, tag="pk",
                            name=f"pk_c{bb + 1}",
                        )
                        gather_pk(pk_tiles[bb + 1], bb + 1)

                    # scores phase: [128 slots, 9 cols]
                    scores_b = sm.tile([128, NCOL], F32, tag="scores")
                    for c in range(NCOL):
                        t_t = tp.tile([128, H], F32)
                        nc.vector.tensor_add(
                            out=t_t, in0=pk_c[:, c, :], in1=q_bc[bb]
                        )
                        nc.scalar.activation(
                            out=t_t, in_=t_t, func=mybir.ActivationFunctionType.Tanh
                        )
                        nc.vector.tensor_mul(out=t_t, in0=t_t, in1=ve_bcast)
                        # weighted reduce over h: Identity activation with
                        # free-dim accumulator (keeps the reduce off DVE)
                        nc.scalar.activation(
                            out=t_t,
                            in_=t_t,
                            func=mybir.ActivationFunctionType.Identity,
                            accum_out=scores_b[:, c : c + 1],
                        )

                    # masked softmax (no max subtraction; |scores| <~ 26).
                    # vmask zeroes the padded slots exactly.
                    e_t = sm.tile([128, NCOL], F32, tag="e")
                    nc.scalar.activation(
                        out=e_t, in_=scores_b, func=mybir.ActivationFunctionType.Exp
                    )
                    nc.vector.tensor_mul(out=e_t, in0=e_t, in1=vmask_t[:, bb, :])
                    rowsum = sm.tile([128, 1], F32, tag="rowsum")
                    nc.vector.reduce_sum(
                        out=rowsum, in_=e_t, axis=mybir.AxisListType.X
                    )

                    # Z broadcast via ones matmul, written into column 0 of
                    # the ctx psum tile (shares the psum banks; the first ctx
                    # matmul resets partition 0 with start=True after recip
                    # has read the column)
                    ctxp = psc.tile([128, VS], F32, tag="ctx")
                    nc.tensor.matmul(
                        out=ctxp[:, 0:1],
                        lhsT=ones128,
                        rhs=rowsum,
                        start=True,
                        stop=True,
                        skip_group_check=True,
                    )
                    recip = sm.tile([128, 1], F32, tag="recip")
                    nc.vector.tensor_copy(out=recip, in_=ctxp[:, 0:1])
                    nc.vector.reciprocal(out=recip, in_=recip)

                    alphas_t = sm.tile([128, NCOL], F32, tag="alphas")
                    nc.vector.tensor_scalar_mul(
                        out=alphas_t, in0=e_t, scalar1=recip
                    )
                    # compacted alphas out; host routes them to full positions
                    nc.gpsimd.dma_start(
                        out=_ap(
                            al_out, bb * 128 * NCOL, [[NCOL, 128], [1, NCOL]]
                        ),
                        in_=alphas_t,
                    )

                    if USE_F32R_CTX:
                        alphas_r = sm.tile([128, NCOL], F32R, tag="alphas_r")
                        nc.vector.tensor_copy(out=alphas_r, in_=alphas_t)
                    else:
                        alphas_r = alphas_t

                    # drain batch bb-1's context psum now: its matmuls
                    # finished while this batch's scores were computing
                    if pending_ctx[bb - 1] is not None:
                        prev_b, prev_ctxp = pending_ctx[bb - 1]
                        ctx_sb = sm.tile([1, VS], F32, tag="ctx_sb")
                        nc.vector.tensor_copy(out=ctx_sb, in_=prev_ctxp[0:1, :])
                        nc.gpsimd.dma_start(
                            out=_ap(ctx_out, prev_b * VS, [[VS, 1], [1, VS]]),
                            in_=ctx_sb,
                        )
                        pending_ctx[bb - 1] = None

                    # context phase: ctx[v] = sum_i alphas_c[i] * v_c[i, v],
                    # alphas-stationary over the 9 compacted columns
                    for g in range(NVG):
                        v_t = v_tiles[g]
                        for cc in range(VCC):
                            col = g * VCC + cc
                            for j in range(VS // 512):
                                nc.tensor.matmul(
                                    out=ctxp[0:1, j * 512 : (j + 1) * 512],
                                    lhsT=alphas_r[:, col : col + 1],
                                    rhs=v_t[:, cc, j * 512 : (j + 1) * 512],
                                    start=(col == 0),
                                    stop=(col == NCOL - 1),
                                    skip_group_check=True,
                                )
                    pending_ctx[bb] = (bb, ctxp)

                last_b, last_ctxp = pending_ctx[BPC - 1]
                ctx_sb = sm.tile([1, VS], F32, tag="ctx_sb")
                nc.vector.tensor_copy(out=ctx_sb, in_=last_ctxp[0:1, :])
                nc.gpsimd.dma_start(
                    out=_ap(ctx_out, last_b * VS, [[VS, 1], [1, VS]]),
                    in_=ctx_sb,
                )

    _split_excess_waits(nc)
    # populate .instr bytes for extended-inst InstISA subclasses (the
    # library-reload op) — raw Bass doesn't run Bacc's codegen pass and the
    # NEFF compiler fails with "ISA wrong length" on empty .instr
    from concourse.library_overlay import lower_extended_insts

    lower_extended_insts(nc)
    return nc


_NC_CACHE = None


def _get_nc():
    global _NC_CACHE
    if _NC_CACHE is None:
        _NC_CACHE = build_nc()
    return _NC_CACHE


def make_core_inputs(k, query, proj_key, value, mask, Wq, v_energy):
    """Host-side shard prep for core k: slice the batch range and build the
    compacted gather index tensors from the mask (pure numpy). Also returns
    the per-batch unmasked position lists for the unshard step."""
    sl = slice(k * BPC, (k + 1) * BPC)
    m = mask[sl, 0, :]  # [BPC, S]
    gidx = np.empty((BPC, 128, NW), np.int16)
    vmask = np.empty((BPC, 128, NCOL), np.float32)
    s_lists = []
    for bb in range(BPC):
        s_list = np.nonzero(m[bb])[0]
        s_lists.append(s_list)
        kcnt = len(s_list)
        assert 0 < kcnt <= NPAD, f"unmasked count {kcnt} outside (0, {NPAD}]"
        flat = np.full(NPAD, bb * S + s_list[0], np.int64)
        flat[:kcnt] = bb * S + s_list
        # dma_gather wraps indices as [16, NW] (slot i -> partition i%16,
        # column i//16), replicated down all 128 partitions
        gidx[bb] = np.tile(flat.reshape(NW, 16).T.astype(np.int16), (8, 1))
        # compacted slot i lives at [partition i%128, column i//128]
        vmask[bb] = (np.arange(NPAD) < kcnt).reshape(NCOL, 128).T.astype(np.float32)
    in_map = {
        "query": query[sl],
        "proj_key": proj_key[sl],
        "value": value[sl],
        "Wq": Wq,
        "v_energy": v_energy,
        "gidx": gidx,
        "vmask": vmask,
    }
    return in_map, s_lists


def unshard_alphas(al_c_per_core, s_lists_per_core):
    """Place compacted alphas [BPC, 128, NCOL] at their full-domain positions.
    Masked positions are exactly 0 (matching softmax over -inf scores)."""
    al = np.zeros((B, 1, S), np.float32)
    for k in range(NCORES):
        al_c = al_c_per_core[k]
        for bb in range(BPC):
            s_list = s_lists_per_core[k][bb]
            # slot i = c*128 + p lives at al_c[bb, p, c]
            slots = al_c[bb].T.ravel()
            al[k * BPC + bb, 0, s_list] = slots[: len(s_list)]
    return al


def kernel(query, proj_key, value, mask, Wq, v_energy, _want_results_obj=False,
           _trace=False):
    query = np.asarray(query, dtype=np.float32)
    proj_key = np.asarray(proj_key, dtype=np.float32)
    value = np.asarray(value, dtype=np.float32)
    mask = np.asarray(mask, dtype=np.int32)
    Wq = np.asarray(Wq, dtype=np.float32)
    v_energy = np.asarray(v_energy, dtype=np.float32)

    nc = _get_nc()
    in_maps, s_lists = [], []
    for k in range(NCORES):
        im, sls = make_core_inputs(k, query, proj_key, value, mask, Wq, v_energy)
        in_maps.append(im)
        s_lists.append(sls)
    res = run_bass_kernel_spmd(
        nc, in_maps, core_ids=list(range(NCORES)), trace=_trace
    )
    ctx = np.concatenate([r["context"] for r in res.results], axis=0)
    al = unshard_alphas([r["alphas_c"] for r in res.results], s_lists)
    if _want_results_obj:
        return (ctx, al), res
    return ctx, al
